# revision 8
# baseline (speedup 1.0000x reference)
"""MoC sparse attention (nn_MoCAttention) on 8 Trainium2 NeuronCores.

Strategy (head-parallel): one attention head per core; host sums the 8
per-head partial outputs.

v3 (vs v1 baseline 120.8us, v2 138.5us):
  - DMA consolidation: the per-dma_start issue cost (~0.6us of engine
    time each, plus in-flight caps) dominated the head in v1/v2. Host
    pre-swizzles x into the SBUF layout [128p, 4d, 2048s] so each
    query-block is ONE 512KB DMA (8 x-DMAs total), weights are packed
    into 6 DMAs, and each query block's output is ONE 512KB DMA.
  - Host pre-splits x into exact f16 hi/lo halves (xhi + xlo == x in
    fp32): same input bytes, no on-device casts/subs.
  - Host precomputes the per-chunk sums of x (fp32): the routing path
    (cskT -> M) unblocks as soon as the small weights land, instead of
    waiting for a 20us on-device reduction of all of x.
  - HAM-aware scheduling: PE spin matmuls bridge the preamble; per-qb
    proj/V/sims chase the DMA so the PE stream stays dense (the PE
    clock-gate drops to 1.2 GHz after ~3.4us of idle).
  - Routing sims in [query, chunk] layout with x-tiles stationary and
    [M_hi|M_lo] moving: no PE transposes of sims.
  - V projected directly in [s, hd] layout: no PE transposes for V.
  - Masked dense attention (as v1): top-5 chunk mask folded into the
    scores contraction; exp on ScalarE only (its 35us is the co-
    bottleneck with the PE's 33us in this phase).
  - Normalization: denominator row -> per-query column via K=1
    matmuls, 128-lane reciprocal, folded into the PSUM->f16 output
    copy (tensor_scalar).
"""
import sys

sys.path.insert(0, "/opt/trn_rl_repo")

import numpy as np

import concourse.bass as bass  # noqa: F401  (registers types)
import concourse.mybir as mybir
import concourse.tile as tile
from concourse import bacc, bass_utils

H = 8
S = 2048
D = 512
HD = 64
CHUNK = 64
C = 32  # number of chunks
TOPK = 5
SCALE = HD ** -0.5
BIG = 1.0e4

NQB = 4          # query blocks of 512
QB = S // NQB    # 512
NT = 16          # s-tiles of 128
NKT = 16         # key tiles of 128
NDT = 4          # d-chunks of 128
KG = 2           # key tiles per scores/exp group

SPIN1 = 14       # pre-data PE warm spins

f32 = mybir.dt.float32
f16 = mybir.dt.float16
AF = mybir.ActivationFunctionType
Alu = mybir.AluOpType


def _emit(nc, tc, xhi_d, xlo_d, wqkv_d, wk32_d, xsum_d, wqT_d, wo_d, eoh_d,
          id16_d, out_d):
    def pool(name, bufs, space="SBUF"):
        return tc.tile_pool(name=name, bufs=bufs, space=space)

    with (
        pool("persist", 1) as persist,
        pool("weights", 1) as weights,
    ):
        # ---- persistent SBUF tensors (x in [p, d, s] swizzled layout)
        xhi = persist.tile([128, NDT, S], f16, name="xhi")
        xlo = persist.tile([128, NDT, S], f16, name="xlo")
        KM = persist.tile([96, S], f16, name="KM")
        QMs = [persist.tile([96, QB], f16, name=f"QM{qb}", tag=f"QM{qb}")
               for qb in range(NQB)]
        V_aug = persist.tile([128, NKT, HD + 1], f16, name="V_aug")
        MVV = [persist.tile([128, 2 * C], f16, name=f"MVV{d}", tag=f"MVV{d}")
               for d in range(NDT)]
        cskT = persist.tile([HD, C], f32, name="cskT")
        notm = [persist.tile([128, NT // NQB, C], f16, name=f"notm{qb}",
                             tag=f"notm{qb}") for qb in range(NQB)]
        ident = persist.tile([128, 128], f16, name="ident")
        warm = persist.tile([128, 128], f16, name="warm")

        wqkv = weights.tile([128, NDT, 192], f16, name="wqkv")
        wk32 = weights.tile([128, NDT, HD], f32, name="wk32")
        xsum = weights.tile([128, NDT, C], f32, name="xsum")
        wqT32 = weights.tile([HD, D], f32, name="wqT32")
        wo16 = weights.tile([HD, D], f16, name="wo16")

        # ---- PE warm-up spins on a memset tile (no DMA dependency)
        nc.vector.memset(warm, 0.0)
        ones16 = persist.tile([128, NKT], f32, name="ones16")
        nc.vector.memset(ones16, 1.0)
        nc.vector.tensor_copy(out=V_aug[:, :, HD], in_=ones16)
        ones65 = persist.tile([HD + 1, 1], f16, name="ones65")
        nc.vector.memset(ones65, 1.0)

        # ---- weight DMAs on the scalar ring; routing inputs (xsum, wk32,
        # wqT32) first so cskT/M unblock early
        nc.scalar.dma_start(out=xsum, in_=xsum_d)
        nc.scalar.dma_start(out=wk32, in_=wk32_d)
        nc.scalar.dma_start(out=wqT32, in_=wqT_d)
        nc.scalar.dma_start(out=wqkv, in_=wqkv_d)
        nc.scalar.dma_start(out=wo16, in_=wo_d)
        nc.scalar.dma_start(out=ident, in_=id16_d)
        nc.scalar.dma_start(out=KM[HD:96, :], in_=eoh_d)

        # ---- x DMAs: per-d slices are 4KB-contiguous per partition row
        # (per-qb slices would be 1KB runs at ~half the DMA rate)
        nc.sync.dma_start(out=xhi[:, 0, :], in_=xhi_d[:, 0, :])
        nc.sync.dma_start(out=xhi[:, 1, :], in_=xhi_d[:, 1, :])
        nc.gpsimd.dma_start(out=xlo[:, 0, :], in_=xlo_d[:, 0, :])
        nc.gpsimd.dma_start(out=xlo[:, 1, :], in_=xlo_d[:, 1, :])
        nc.scalar.dma_start(out=xhi[:, 2, :], in_=xhi_d[:, 2, :])
        nc.scalar.dma_start(out=xhi[:, 3, :], in_=xhi_d[:, 3, :])
        nc.sync.dma_start(out=xlo[:, 2, :], in_=xlo_d[:, 2, :])
        nc.sync.dma_start(out=xlo[:, 3, :], in_=xlo_d[:, 3, :])

        with pool("ps_warm", 1, space="PSUM") as ps_warm:
            p_warm = ps_warm.tile([128, 128], f32, name="p_warm")
            for _ in range(SPIN1):
                nc.tensor.matmul(p_warm, warm, warm, start=True, stop=True)

        # ---- routing M: cskT = Wk^T @ xsum, M_d = Wq_d^T @ cskT
        # (xsum precomputed; ready as soon as the small DMAs land)
        with pool("ps_rt", 2, space="PSUM") as ps_rt:
            p_csk = ps_rt.tile([HD, C], f32, name="p_csk", tag="p_small")
            for d in range(NDT):
                nc.tensor.matmul(p_csk, wk32[:, d, :], xsum[:, d, :],
                                 start=(d == 0), stop=(d == NDT - 1))
            nc.vector.tensor_copy(out=cskT, in_=p_csk)
            for d in range(NDT):
                p_m = ps_rt.tile([128, C], f32, name="p_m", tag="p_small")
                nc.tensor.matmul(p_m, wqT32[:, d * 128:(d + 1) * 128], cskT,
                                 start=True, stop=True)
                # MVV[d] = [M_hi | M_lo] (f16 hi/lo split of M col block)
                nc.vector.tensor_copy(out=MVV[d][:, 0:C], in_=p_m)
                nc.vector.tensor_sub(out=MVV[d][:, C:2 * C], in0=p_m,
                                     in1=MVV[d][:, 0:C])

        # ---- per query block: QK proj, V proj, sims+mask (chases DMA)
        with (
            pool("ps_qk", 2, space="PSUM") as ps_qk,
            pool("ps_v", 2, space="PSUM") as ps_v,
            pool("ps_sims", 3, space="PSUM") as ps_sims,
            pool("ps_nmT", 1, space="PSUM") as ps_nmT,
            pool("rt_sb", 6) as rt_sb,
        ):
            for qb in range(NQB):
                sl = slice(qb * QB, (qb + 1) * QB)
                p_qk = ps_qk.tile([128, QB], f32, name="p_qk", tag="p_qk")
                for d in range(NDT):
                    nc.tensor.matmul(p_qk, wqkv[:, d, 0:128], xhi[:, d, sl],
                                     start=(d == 0), stop=(d == NDT - 1))
                nc.vector.tensor_copy(out=QMs[qb][0:HD, :], in_=p_qk[0:HD, :])
                nc.vector.tensor_copy(out=KM[0:HD, sl], in_=p_qk[HD:128, :])
                for j in range(NT // NQB):
                    st = qb * (NT // NQB) + j
                    tsl = slice(st * 128, (st + 1) * 128)
                    # V tile direct in [s, hd]: x-tile stationary
                    p_v = ps_v.tile([128, HD], f32, name="p_v", tag="p_v")
                    for d in range(NDT):
                        nc.tensor.matmul(p_v, xhi[:, d, tsl],
                                         wqkv[:, d, 128:192],
                                         start=(d == 0), stop=(d == NDT - 1))
                    nc.vector.tensor_copy(out=V_aug[:, st, 0:HD], in_=p_v)
                    # sims tile [128q, 64]: cols 0:32 hi terms + xlo.M_hi,
                    # cols 32:64 xhi.M_lo
                    p_t = ps_sims.tile([128, 2 * C], f32, name="p_t",
                                       tag="p_t")
                    nc.tensor.matmul(p_t, xhi[:, 0, tsl], MVV[0],
                                     start=True, stop=False,
                                     skip_group_check=True)
                    for d in range(NDT):
                        nc.tensor.matmul(p_t[:, 0:C], xlo[:, d, tsl],
                                         MVV[d][:, 0:C], start=False,
                                         stop=False, skip_group_check=True)
                    for d in range(1, NDT):
                        nc.tensor.matmul(p_t, xhi[:, d, tsl], MVV[d],
                                         start=False, stop=(d == NDT - 1),
                                         skip_group_check=True)
                    sims = rt_sb.tile([128, C], f32, name="sims", tag="sims")
                    nc.vector.tensor_copy(out=sims, in_=p_t[:, C:2 * C])
                    nc.vector.tensor_add(out=sims, in0=sims, in1=p_t[:, 0:C])
                    top8 = rt_sb.tile([128, 8], f32, name="top8", tag="top8")
                    nc.vector.max(out=top8, in_=sims)
                    nc.vector.tensor_scalar(
                        out=notm[qb][:, j, :], in0=sims,
                        scalar1=top8[:, TOPK - 1:TOPK],
                        scalar2=None, op0=Alu.is_lt)
                # transpose notmask [128q, (t,c)] -> [(t,c), 128q] bias rows
                p_nmT = ps_nmT.tile([128, 128], f16, name="p_nmT",
                                    tag="p_nmT")
                nc.tensor.transpose(p_nmT, notm[qb], ident)
                for j in range(NT // NQB):
                    nc.vector.tensor_scalar_mul(
                        out=QMs[qb][HD:96, j * 128:(j + 1) * 128],
                        in0=p_nmT[j * C:(j + 1) * C, :], scalar1=-BIG)

        # ---- masked dense attention + normalize + Wo + store
        with (
            pool("ps_sc", 2, space="PSUM") as ps_sc,
            pool("ps_pv", 2, space="PSUM") as ps_pv,
            pool("ps_tail", 2, space="PSUM") as ps_tail,
            pool("exp_sb", 3) as exp_sb,
            pool("tail_sb", 4) as tail_sb,
            pool("out_sb", 2) as out_sb_pool,
        ):
            out_rings = [nc.sync, nc.gpsimd]
            pvs = [None] * NQB

            def emit_tail(qb):
                # numerator + denominator row -> SBUF f16; den row ->
                # per-query column via K=1 matmuls; reciprocal; Wo;
                # normalize folded into the PSUM->f16 copy
                p_pv = pvs[qb]
                outTa = tail_sb.tile([HD + 1, QB], f16, name="outTa",
                                     tag="outTa")
                nc.vector.tensor_copy(out=outTa, in_=p_pv)
                p_dn = ps_tail.tile([128, 4], f32, name="p_dn", tag="p_tail")
                for j in range(4):
                    nc.tensor.matmul(
                        p_dn[:, j:j + 1],
                        outTa[HD:HD + 1, j * 128:(j + 1) * 128],
                        ones65[HD:HD + 1, 0:1], start=True, stop=True)
                rdenT = tail_sb.tile([128, 4], f32, name="rdenT", tag="rdenT")
                nc.vector.reciprocal(out=rdenT, in_=p_dn)
                o16q = out_sb_pool.tile([128, 4, D], f16, name="o16q",
                                        tag="o16q")
                for j in range(4):
                    p_wo = ps_tail.tile([128, D], f32, name="p_wo",
                                        tag="p_tail")
                    nc.tensor.matmul(p_wo,
                                     outTa[0:HD, j * 128:(j + 1) * 128],
                                     wo16, start=True, stop=True)
                    if qb == NQB - 1 and j % 2 == 1:
                        # final block: split normalize with ScalarE (exp
                        # stream is over by then) to shorten the tail
                        nc.scalar.activation(out=o16q[:, j, :], in_=p_wo,
                                             func=AF.Copy,
                                             scale=rdenT[:, j:j + 1])
                    else:
                        nc.vector.tensor_scalar(
                            out=o16q[:, j, :], in0=p_wo,
                            scalar1=rdenT[:, j:j + 1], scalar2=None,
                            op0=Alu.mult)
                out_rings[qb % 2].dma_start(
                    out=out_d[qb * QB:(qb + 1) * QB, :].rearrange(
                        "(j p) n -> p j n", p=128),
                    in_=o16q)

            for qb in range(NQB):
                p_pv = ps_pv.tile([HD + 1, QB], f32, name="p_pv", tag="p_pv")
                pvs[qb] = p_pv
                for g in range(NKT // KG):
                    p_sc = ps_sc.tile([128, KG * QB], f32, name="p_sc",
                                      tag="p_sc")
                    for i in range(KG):
                        kt = KG * g + i
                        nc.tensor.matmul(
                            p_sc[:, i * QB:(i + 1) * QB],
                            KM[:, kt * 128:(kt + 1) * 128], QMs[qb],
                            start=True, stop=True)
                    expT = exp_sb.tile([128, KG * QB], f16, name="expT",
                                       tag="expT")
                    nc.scalar.activation(out=expT, in_=p_sc, func=AF.Exp,
                                         scale=SCALE)
                    for i in range(KG):
                        kt = KG * g + i
                        nc.tensor.matmul(
                            p_pv, V_aug[:, kt, :],
                            expT[:, i * QB:(i + 1) * QB],
                            start=(kt == 0), stop=(kt == NKT - 1))
                    # previous block's tail goes after this block's first
                    # scores/exp so the exp stream never starves
                    if g == 1 and qb > 0:
                        emit_tail(qb - 1)
                if qb == NQB - 1:
                    emit_tail(qb)


_CACHED_NC = None


def _build():
    global _CACHED_NC
    if _CACHED_NC is not None:
        return _CACHED_NC
    nc = bacc.Bacc("TRN2", target_bir_lowering=False, debug=False)
    xhi_d = nc.dram_tensor("xhi", [128, NDT, S], f16, kind="ExternalInput").ap()
    xlo_d = nc.dram_tensor("xlo", [128, NDT, S], f16, kind="ExternalInput").ap()
    wqkv_d = nc.dram_tensor("wqkv", [128, NDT, 192], f16,
                            kind="ExternalInput").ap()
    wk32_d = nc.dram_tensor("wk32", [128, NDT, HD], f32,
                            kind="ExternalInput").ap()
    xsum_d = nc.dram_tensor("xsum", [128, NDT, C], f32,
                            kind="ExternalInput").ap()
    wqT_d = nc.dram_tensor("wqT", [HD, D], f32, kind="ExternalInput").ap()
    wo_d = nc.dram_tensor("wo", [HD, D], f16, kind="ExternalInput").ap()
    eoh_d = nc.dram_tensor("eoh", [C, S], f16, kind="ExternalInput").ap()
    id16_d = nc.dram_tensor("id16", [128, 128], f16, kind="ExternalInput").ap()
    out_d = nc.dram_tensor("out", [S, D], f16, kind="ExternalOutput").ap()
    with tile.TileContext(nc) as tc:
        _emit(nc, tc, xhi_d, xlo_d, wqkv_d, wk32_d, xsum_d, wqT_d, wo_d,
              eoh_d, id16_d, out_d)
    nc.compile()
    _CACHED_NC = nc
    return nc


def _swz(a):
    """[512, X] feature-major -> [128, NDT, X] (p, d, X) SBUF layout."""
    return np.ascontiguousarray(
        a.reshape(NDT, 128, a.shape[1]).transpose(1, 0, 2))


def _in_maps(x, Wq, Wk, Wv, Wo):
    x = np.asarray(x, dtype=np.float32).reshape(S, D)
    Wq = np.asarray(Wq, dtype=np.float32)
    Wk = np.asarray(Wk, dtype=np.float32)
    Wv = np.asarray(Wv, dtype=np.float32)
    Wo = np.asarray(Wo, dtype=np.float32)
    xhi = x.astype(np.float16)
    xlo = (x - xhi.astype(np.float32)).astype(np.float16)
    xhi_s = _swz(np.ascontiguousarray(xhi.T))
    xlo_s = _swz(np.ascontiguousarray(xlo.T))
    # exact-ish per-chunk sums of x (fp64 accum, fp32 store): [D, C]
    xsum = x.reshape(C, CHUNK, D).astype(np.float64).sum(axis=1)
    xsum_s = _swz(np.ascontiguousarray(xsum.T.astype(np.float32)))
    eoh = np.kron(np.eye(C, dtype=np.float16), np.ones((1, CHUNK), np.float16))
    eoh = np.ascontiguousarray(eoh)
    ident16 = np.eye(128, dtype=np.float16)
    maps = []
    for h in range(H):
        sl = slice(HD * h, HD * (h + 1))
        wqkv = np.concatenate([Wq[:, sl], Wk[:, sl], Wv[:, sl]],
                              axis=1).astype(np.float16)
        maps.append({
            "xhi": xhi_s,
            "xlo": xlo_s,
            "wqkv": _swz(np.ascontiguousarray(wqkv)),
            "wk32": _swz(np.ascontiguousarray(Wk[:, sl])),
            "xsum": xsum_s,
            "wqT": np.ascontiguousarray(Wq[:, sl].T),
            "wo": np.ascontiguousarray(Wo[sl, :]).astype(np.float16),
            "eoh": eoh,
            "id16": ident16,
        })
    return maps


def _ensure_profile_hook():
    """Register antenv.axon_hooks (NTFF profiling shim) if missing."""
    import importlib.util
    if importlib.util.find_spec("antenv.axon_hooks") is not None:
        return
    import importlib.machinery
    import antenv
    path = "/opt/trn_rl_repo/antenv/axon_hooks.py"
    loader = importlib.machinery.SourceFileLoader("antenv.axon_hooks", path)
    spec = importlib.util.spec_from_loader(loader.name, loader)
    mod = importlib.util.module_from_spec(spec)
    loader.exec_module(mod)
    sys.modules["antenv.axon_hooks"] = mod
    antenv.axon_hooks = mod


def run(x, Wq, Wk, Wv, Wo, trace=False):
    if trace:
        _ensure_profile_hook()
    nc = _build()
    res = bass_utils.run_bass_kernel_spmd(
        nc, _in_maps(x, Wq, Wk, Wv, Wo), core_ids=list(range(H)), trace=trace)
    acc = np.zeros((S, D), dtype=np.float64)
    for r in res.results:
        acc += r["out"].astype(np.float64)
    return acc.astype(np.float32).reshape(1, S, D), res


def kernel(x, Wq, Wk, Wv, Wo):
    out, _ = run(x, Wq, Wk, Wv, Wo)
    return out


# revision 12
# speedup vs baseline: 1.0437x; 1.0437x over previous
"""MoC sparse attention (nn_MoCAttention) on 8 Trainium2 NeuronCores.

Strategy (head-parallel): one attention head per core; host sums the 8
per-head partial outputs.

v3 (vs v1 baseline 120.8us, v2 138.5us):
  - DMA consolidation: the per-dma_start issue cost (~0.6us of engine
    time each, plus in-flight caps) dominated the head in v1/v2. Host
    pre-swizzles x into the SBUF layout [128p, 4d, 2048s] so each
    query-block is ONE 512KB DMA (8 x-DMAs total), weights are packed
    into 6 DMAs, and each query block's output is ONE 512KB DMA.
  - Host pre-splits x into exact f16 hi/lo halves (xhi + xlo == x in
    fp32): same input bytes, no on-device casts/subs.
  - Host precomputes the per-chunk sums of x (fp32): the routing path
    (cskT -> M) unblocks as soon as the small weights land, instead of
    waiting for a 20us on-device reduction of all of x.
  - HAM-aware scheduling: PE spin matmuls bridge the preamble; per-qb
    proj/V/sims chase the DMA so the PE stream stays dense (the PE
    clock-gate drops to 1.2 GHz after ~3.4us of idle).
  - Routing sims in [query, chunk] layout with x-tiles stationary and
    [M_hi|M_lo] moving: no PE transposes of sims.
  - V projected directly in [s, hd] layout: no PE transposes for V.
  - Masked dense attention (as v1): top-5 chunk mask folded into the
    scores contraction; exp on ScalarE only (its 35us is the co-
    bottleneck with the PE's 33us in this phase).
  - Normalization: denominator row -> per-query column via K=1
    matmuls, 128-lane reciprocal, folded into the PSUM->f16 output
    copy (tensor_scalar).
"""
import sys

sys.path.insert(0, "/opt/trn_rl_repo")

import numpy as np

import concourse.bass as bass  # noqa: F401  (registers types)
import concourse.mybir as mybir
import concourse.tile as tile
from concourse import bacc, bass_utils

H = 8
S = 2048
D = 512
HD = 64
CHUNK = 64
C = 32  # number of chunks
TOPK = 5
SCALE = HD ** -0.5
BIG = 1.0e4

NQB = 4          # query blocks of 512
QB = S // NQB    # 512
NT = 16          # s-tiles of 128
NKT = 16         # key tiles of 128
NDT = 4          # d-chunks of 128
KG = 2           # key tiles per scores/exp group

SPIN1 = 14       # pre-data PE warm spins

f32 = mybir.dt.float32
f16 = mybir.dt.float16
AF = mybir.ActivationFunctionType
Alu = mybir.AluOpType


def _emit(nc, tc, xhi_d, xlo_d, wqkv_d, wk32_d, xsum_d, wqT_d, wo_d, eoh_d,
          id16_d, out_d):
    def pool(name, bufs, space="SBUF"):
        return tc.tile_pool(name=name, bufs=bufs, space=space)

    with (
        pool("persist", 1) as persist,
        pool("weights", 1) as weights,
    ):
        # ---- persistent SBUF tensors (x in [p, d, s] swizzled layout)
        xhi = persist.tile([128, NDT, S], f16, name="xhi")
        xlo = persist.tile([128, NDT, S], f16, name="xlo")
        KM = persist.tile([96, S], f16, name="KM")
        QMs = [persist.tile([96, QB], f16, name=f"QM{qb}", tag=f"QM{qb}")
               for qb in range(NQB)]
        V_aug = persist.tile([128, NKT, HD + 1], f16, name="V_aug")
        MVV = [persist.tile([128, 2 * C], f16, name=f"MVV{d}", tag=f"MVV{d}")
               for d in range(NDT)]
        cskT = persist.tile([HD, C], f32, name="cskT")
        notm = [persist.tile([128, NT // NQB, C], f16, name=f"notm{qb}",
                             tag=f"notm{qb}") for qb in range(NQB)]
        ident = persist.tile([128, 128], f16, name="ident")
        warm = persist.tile([128, 128], f16, name="warm")

        wqkv = weights.tile([128, NDT, 192], f16, name="wqkv")
        wk32 = weights.tile([128, NDT, HD], f32, name="wk32")
        xsum = weights.tile([128, NDT, C], f32, name="xsum")
        wqT32 = weights.tile([HD, D], f32, name="wqT32")
        wo16 = weights.tile([HD, D], f16, name="wo16")

        # ---- PE warm-up spins on a memset tile (no DMA dependency)
        nc.vector.memset(warm, 0.0)
        ones16 = persist.tile([128, NKT], f32, name="ones16")
        nc.vector.memset(ones16, 1.0)
        nc.vector.tensor_copy(out=V_aug[:, :, HD], in_=ones16)
        ones65 = persist.tile([HD + 1, 1], f16, name="ones65")
        nc.vector.memset(ones65, 1.0)

        # ---- weight DMAs on the scalar ring; routing inputs (xsum, wk32,
        # wqT32) first so cskT/M unblock early
        nc.scalar.dma_start(out=xsum, in_=xsum_d)
        nc.scalar.dma_start(out=wk32, in_=wk32_d)
        nc.scalar.dma_start(out=wqT32, in_=wqT_d)
        nc.scalar.dma_start(out=wqkv, in_=wqkv_d)
        nc.scalar.dma_start(out=wo16, in_=wo_d)
        nc.scalar.dma_start(out=ident, in_=id16_d)
        nc.scalar.dma_start(out=KM[HD:96, :], in_=eoh_d)

        # ---- x DMAs: per-d slices are 4KB-contiguous per partition row
        # (per-qb slices would be 1KB runs at ~half the DMA rate)
        nc.sync.dma_start(out=xhi[:, 0, :], in_=xhi_d[:, 0, :])
        nc.gpsimd.dma_start(out=xhi[:, 1, :], in_=xhi_d[:, 1, :])
        nc.sync.dma_start(out=xhi[:, 2, :], in_=xhi_d[:, 2, :])
        nc.gpsimd.dma_start(out=xhi[:, 3, :], in_=xhi_d[:, 3, :])
        nc.sync.dma_start(out=xlo[:, 0, :], in_=xlo_d[:, 0, :])
        nc.gpsimd.dma_start(out=xlo[:, 1, :], in_=xlo_d[:, 1, :])
        nc.sync.dma_start(out=xlo[:, 2, :], in_=xlo_d[:, 2, :])
        nc.gpsimd.dma_start(out=xlo[:, 3, :], in_=xlo_d[:, 3, :])

        with pool("ps_warm", 1, space="PSUM") as ps_warm:
            p_warm = ps_warm.tile([128, 128], f32, name="p_warm")
            for _ in range(SPIN1):
                nc.tensor.matmul(p_warm, warm, warm, start=True, stop=True)

        # ---- routing M: cskT = Wk^T @ xsum, M_d = Wq_d^T @ cskT
        # (xsum precomputed; ready as soon as the small DMAs land)
        with pool("ps_rt", 2, space="PSUM") as ps_rt:
            p_csk = ps_rt.tile([HD, C], f32, name="p_csk", tag="p_small")
            for d in range(NDT):
                nc.tensor.matmul(p_csk, wk32[:, d, :], xsum[:, d, :],
                                 start=(d == 0), stop=(d == NDT - 1))
            nc.vector.tensor_copy(out=cskT, in_=p_csk)
            for d in range(NDT):
                p_m = ps_rt.tile([128, C], f32, name="p_m", tag="p_small")
                nc.tensor.matmul(p_m, wqT32[:, d * 128:(d + 1) * 128], cskT,
                                 start=True, stop=True)
                # MVV[d] = [M_hi | M_lo] (f16 hi/lo split of M col block)
                nc.vector.tensor_copy(out=MVV[d][:, 0:C], in_=p_m)
                nc.vector.tensor_sub(out=MVV[d][:, C:2 * C], in0=p_m,
                                     in1=MVV[d][:, 0:C])

        # ---- per query block: QK proj, V proj, sims+mask (chases DMA)
        with (
            pool("ps_qk", 2, space="PSUM") as ps_qk,
            pool("ps_v", 2, space="PSUM") as ps_v,
            pool("ps_sims", 3, space="PSUM") as ps_sims,
            pool("ps_nmT", 1, space="PSUM") as ps_nmT,
            pool("rt_sb", 6) as rt_sb,
        ):
            for qb in range(NQB):
                sl = slice(qb * QB, (qb + 1) * QB)
                p_qk = ps_qk.tile([128, QB], f32, name="p_qk", tag="p_qk")
                for d in range(NDT):
                    nc.tensor.matmul(p_qk, wqkv[:, d, 0:128], xhi[:, d, sl],
                                     start=(d == 0), stop=(d == NDT - 1))
                nc.vector.tensor_copy(out=QMs[qb][0:HD, :], in_=p_qk[0:HD, :])
                nc.scalar.copy(out=KM[0:HD, sl], in_=p_qk[HD:128, :])
                for j in range(NT // NQB):
                    st = qb * (NT // NQB) + j
                    tsl = slice(st * 128, (st + 1) * 128)
                    # V tile direct in [s, hd]: x-tile stationary
                    p_v = ps_v.tile([128, HD], f32, name="p_v", tag="p_v")
                    for d in range(NDT):
                        nc.tensor.matmul(p_v, xhi[:, d, tsl],
                                         wqkv[:, d, 128:192],
                                         start=(d == 0), stop=(d == NDT - 1))
                    nc.scalar.copy(out=V_aug[:, st, 0:HD], in_=p_v)
                    # sims tile [128q, 64]: cols 0:32 hi terms + xlo.M_hi,
                    # cols 32:64 xhi.M_lo
                    p_t = ps_sims.tile([128, 2 * C], f32, name="p_t",
                                       tag="p_t")
                    nc.tensor.matmul(p_t, xhi[:, 0, tsl], MVV[0],
                                     start=True, stop=False,
                                     skip_group_check=True)
                    for d in range(NDT):
                        nc.tensor.matmul(p_t[:, 0:C], xlo[:, d, tsl],
                                         MVV[d][:, 0:C], start=False,
                                         stop=False, skip_group_check=True)
                    for d in range(1, NDT):
                        nc.tensor.matmul(p_t, xhi[:, d, tsl], MVV[d],
                                         start=False, stop=(d == NDT - 1),
                                         skip_group_check=True)
                    sims = rt_sb.tile([128, C], f32, name="sims", tag="sims")
                    nc.vector.tensor_copy(out=sims, in_=p_t[:, C:2 * C])
                    nc.vector.tensor_add(out=sims, in0=sims, in1=p_t[:, 0:C])
                    top8 = rt_sb.tile([128, 8], f32, name="top8", tag="top8")
                    nc.vector.max(out=top8, in_=sims)
                    nc.vector.tensor_scalar(
                        out=notm[qb][:, j, :], in0=sims,
                        scalar1=top8[:, TOPK - 1:TOPK],
                        scalar2=None, op0=Alu.is_lt)
                # transpose notmask [128q, (t,c)] -> [(t,c), 128q] bias rows
                p_nmT = ps_nmT.tile([128, 128], f16, name="p_nmT",
                                    tag="p_nmT")
                nc.tensor.transpose(p_nmT, notm[qb], ident)
                for j in range(NT // NQB):
                    nc.vector.tensor_scalar_mul(
                        out=QMs[qb][HD:96, j * 128:(j + 1) * 128],
                        in0=p_nmT[j * C:(j + 1) * C, :], scalar1=-BIG)

        # ---- masked dense attention + normalize + Wo + store
        with (
            pool("ps_sc", 2, space="PSUM") as ps_sc,
            pool("ps_pv", 2, space="PSUM") as ps_pv,
            pool("ps_tail", 2, space="PSUM") as ps_tail,
            pool("exp_sb", 3) as exp_sb,
            pool("tail_sb", 4) as tail_sb,
            pool("out_sb", 2) as out_sb_pool,
        ):
            out_rings = [nc.sync, nc.gpsimd]
            state = {}

            def tail_stage(qb, stage):
                # previous block's epilogue, one small piece per group
                # boundary so the exp stream never starves:
                # stage 0: numerator+den -> SBUF f16; den row -> column
                #          via K=1 matmuls; reciprocal
                # stage 1+j: Wo matmul for query tile j; normalize folded
                #          into the PSUM->f16 copy
                # stage 5: output DMA (one 512KB transfer)
                if stage == 0:
                    outTa = tail_sb.tile([HD + 1, QB], f16, name="outTa",
                                         tag="outTa")
                    nc.vector.tensor_copy(out=outTa, in_=state['pv'])
                    p_dn = ps_tail.tile([128, 4], f32, name="p_dn",
                                        tag="p_tail")
                    for j in range(4):
                        nc.tensor.matmul(
                            p_dn[:, j:j + 1],
                            outTa[HD:HD + 1, j * 128:(j + 1) * 128],
                            ones65[HD:HD + 1, 0:1], start=True, stop=True)
                    rdenT = tail_sb.tile([128, 4], f32, name="rdenT",
                                         tag="rdenT")
                    nc.vector.reciprocal(out=rdenT, in_=p_dn)
                    o16q = out_sb_pool.tile([128, 4, D], f16, name="o16q",
                                            tag="o16q")
                    state.update(outTa=outTa, rdenT=rdenT, o16q=o16q)
                elif stage <= 4:
                    j = stage - 1
                    p_wo = ps_tail.tile([128, D], f32, name="p_wo",
                                        tag="p_tail")
                    nc.tensor.matmul(
                        p_wo, state['outTa'][0:HD, j * 128:(j + 1) * 128],
                        wo16, start=True, stop=True)
                    nc.vector.tensor_scalar(
                        out=state['o16q'][:, j, :], in0=p_wo,
                        scalar1=state['rdenT'][:, j:j + 1], scalar2=None,
                        op0=Alu.mult)
                else:
                    nc.gpsimd.dma_start(
                        out=out_d[qb * QB:(qb + 1) * QB, :].rearrange(
                            "(j p) n -> p j n", p=128),
                        in_=state['o16q'])

            for qb in range(NQB):
                p_pv = ps_pv.tile([HD + 1, QB], f32, name="p_pv", tag="p_pv")
                for g in range(NKT // KG):
                    p_sc = ps_sc.tile([128, KG * QB], f32, name="p_sc",
                                      tag="p_sc")
                    for i in range(KG):
                        kt = KG * g + i
                        nc.tensor.matmul(
                            p_sc[:, i * QB:(i + 1) * QB],
                            KM[:, kt * 128:(kt + 1) * 128], QMs[qb],
                            start=True, stop=True)
                    expT = exp_sb.tile([128, KG * QB], f16, name="expT",
                                       tag="expT")
                    nc.scalar.activation(out=expT, in_=p_sc, func=AF.Exp,
                                         scale=SCALE)
                    for i in range(KG):
                        kt = KG * g + i
                        nc.tensor.matmul(
                            p_pv, V_aug[:, kt, :],
                            expT[:, i * QB:(i + 1) * QB],
                            start=(kt == 0), stop=(kt == NKT - 1))
                    if qb > 0 and 1 <= g <= 6:
                        tail_stage(qb - 1, g - 1)
                state['pv'] = p_pv
            for stage in range(6):
                tail_stage(NQB - 1, stage)


_CACHED_NC = None


def _build():
    global _CACHED_NC
    if _CACHED_NC is not None:
        return _CACHED_NC
    nc = bacc.Bacc("TRN2", target_bir_lowering=False, debug=False)
    xhi_d = nc.dram_tensor("xhi", [128, NDT, S], f16, kind="ExternalInput").ap()
    xlo_d = nc.dram_tensor("xlo", [128, NDT, S], f16, kind="ExternalInput").ap()
    wqkv_d = nc.dram_tensor("wqkv", [128, NDT, 192], f16,
                            kind="ExternalInput").ap()
    wk32_d = nc.dram_tensor("wk32", [128, NDT, HD], f32,
                            kind="ExternalInput").ap()
    xsum_d = nc.dram_tensor("xsum", [128, NDT, C], f32,
                            kind="ExternalInput").ap()
    wqT_d = nc.dram_tensor("wqT", [HD, D], f32, kind="ExternalInput").ap()
    wo_d = nc.dram_tensor("wo", [HD, D], f16, kind="ExternalInput").ap()
    eoh_d = nc.dram_tensor("eoh", [C, S], f16, kind="ExternalInput").ap()
    id16_d = nc.dram_tensor("id16", [128, 128], f16, kind="ExternalInput").ap()
    out_d = nc.dram_tensor("out", [S, D], f16, kind="ExternalOutput").ap()
    with tile.TileContext(nc) as tc:
        _emit(nc, tc, xhi_d, xlo_d, wqkv_d, wk32_d, xsum_d, wqT_d, wo_d,
              eoh_d, id16_d, out_d)
    nc.compile()
    _CACHED_NC = nc
    return nc


def _swz(a):
    """[512, X] feature-major -> [128, NDT, X] (p, d, X) SBUF layout."""
    return np.ascontiguousarray(
        a.reshape(NDT, 128, a.shape[1]).transpose(1, 0, 2))


def _in_maps(x, Wq, Wk, Wv, Wo):
    x = np.asarray(x, dtype=np.float32).reshape(S, D)
    Wq = np.asarray(Wq, dtype=np.float32)
    Wk = np.asarray(Wk, dtype=np.float32)
    Wv = np.asarray(Wv, dtype=np.float32)
    Wo = np.asarray(Wo, dtype=np.float32)
    xhi = x.astype(np.float16)
    xlo = (x - xhi.astype(np.float32)).astype(np.float16)
    xhi_s = _swz(np.ascontiguousarray(xhi.T))
    xlo_s = _swz(np.ascontiguousarray(xlo.T))
    # exact-ish per-chunk sums of x (fp64 accum, fp32 store): [D, C]
    xsum = x.reshape(C, CHUNK, D).astype(np.float64).sum(axis=1)
    xsum_s = _swz(np.ascontiguousarray(xsum.T.astype(np.float32)))
    eoh = np.kron(np.eye(C, dtype=np.float16), np.ones((1, CHUNK), np.float16))
    eoh = np.ascontiguousarray(eoh)
    ident16 = np.eye(128, dtype=np.float16)
    maps = []
    for h in range(H):
        sl = slice(HD * h, HD * (h + 1))
        wqkv = np.concatenate([Wq[:, sl], Wk[:, sl], Wv[:, sl]],
                              axis=1).astype(np.float16)
        maps.append({
            "xhi": xhi_s,
            "xlo": xlo_s,
            "wqkv": _swz(np.ascontiguousarray(wqkv)),
            "wk32": _swz(np.ascontiguousarray(Wk[:, sl])),
            "xsum": xsum_s,
            "wqT": np.ascontiguousarray(Wq[:, sl].T),
            "wo": np.ascontiguousarray(Wo[sl, :]).astype(np.float16),
            "eoh": eoh,
            "id16": ident16,
        })
    return maps


def _ensure_profile_hook():
    """Register antenv.axon_hooks (NTFF profiling shim) if missing."""
    import importlib.util
    if importlib.util.find_spec("antenv.axon_hooks") is not None:
        return
    import importlib.machinery
    import antenv
    path = "/opt/trn_rl_repo/antenv/axon_hooks.py"
    loader = importlib.machinery.SourceFileLoader("antenv.axon_hooks", path)
    spec = importlib.util.spec_from_loader(loader.name, loader)
    mod = importlib.util.module_from_spec(spec)
    loader.exec_module(mod)
    sys.modules["antenv.axon_hooks"] = mod
    antenv.axon_hooks = mod


def run(x, Wq, Wk, Wv, Wo, trace=False):
    if trace:
        _ensure_profile_hook()
    nc = _build()
    res = bass_utils.run_bass_kernel_spmd(
        nc, _in_maps(x, Wq, Wk, Wv, Wo), core_ids=list(range(H)), trace=trace)
    acc = np.zeros((S, D), dtype=np.float64)
    for r in res.results:
        acc += r["out"].astype(np.float64)
    return acc.astype(np.float32).reshape(1, S, D), res


def kernel(x, Wq, Wk, Wv, Wo):
    out, _ = run(x, Wq, Wk, Wv, Wo)
    return out
